# revision 1
# baseline (speedup 1.0000x reference)
"""ALiBi attention (B=4, T=2048, C=672, H=14, D=48) on 8 trn2 NeuronCores.

Key observation: the ALiBi bias max(col-row,0)*slope is exactly zero inside
the causal region (col<=row), so the module reduces to plain causal attention
with scale 1/sqrt(48).

Sharding: core c -> batch b=c//2, head-half s=c%2 (7 of the 14 heads).
Each core computes its heads' attention and a partial output projection
(rows of w_proj for its heads); the host sums the two partials per batch.

Layout strategy per core (all compute in bf16, accum fp32):
  - qkT = [w_q|w_k]^T @ x^T -> [672, 2048] "transposed" feature-major layout,
    repacked per head-pair at partition bases {0, 64}.
  - v in natural [T, 48] layout per head with a ones column appended: the AV
    matmul then yields o^T rows 0..47 and the softmax denominator in row 48.
  - scores s^T[j,i] = k^T.T @ q^T per 128-key tile; exp on ScalarE directly
    from PSUM (no max subtraction needed: |s*scale| <~ 7); causal handled by
    trimming spans to j<=i plus one [128,128] triangular mask multiply.
  - AV accumulates over key tiles in PSUM; heads A/B of a pair share PSUM
    banks at partition bases 0 and 64.
"""

import math
import os
import sys
import types
from contextlib import ExitStack

import numpy as np

if not any(os.path.isdir(os.path.join(p, "concourse")) for p in sys.path):
    sys.path.insert(0, "/opt/trn_rl_repo")

import ml_dtypes  # noqa: E402

import concourse.bass as bass  # noqa: E402
import concourse.mybir as mybir  # noqa: E402
from concourse import tile  # noqa: E402
from concourse.bass_utils import run_bass_kernel_spmd  # noqa: E402

BF16 = ml_dtypes.bfloat16

T = 2048
C = 672
H = 14
D = 48
B = 4
HPC = 7          # heads per core
CH = HPC * D     # 336 per-core head channels
SCALE = 1.0 / math.sqrt(D)

# K tiles over the C=672 contraction dim
KT = [(i * 128, min(128, C - i * 128)) for i in range((C + 127) // 128)]


def _install_tile_drain_patch():
    """walrus CoreV3 allows only one sync-wait per ctrl instruction; the
    TileContext exit drain can carry several. Split them across drains."""
    import concourse.tile as _tile
    from concourse.tile import ScopedClock

    if getattr(_tile.TileContext, "_drain_patched", False):
        return

    def _patched(self, tick_clock, wait_clock):
        drain_inst = self.nc.sync.drain()
        wait_clock.add_sem_waits(
            drain_inst.ins, ScopedClock({None: tick_clock.global_clock})
        )
        si = drain_inst.ins.sync_info
        if si is not None and len(si.on_wait) > 1:
            waits = list(si.on_wait)
            si.on_wait = waits[:1]
            drain_inst.ins.sync_info = si
            for i in range(1, len(waits)):
                extra = self.nc.sync.drain()
                extra.ins.sync_info = mybir.SyncInfo(
                    on_wait=waits[i : i + 1], on_update=[]
                )
        self.nc.all_engine_barrier()
        assert self.sems is not None
        popped = self.nc._tile_sem_poison_stack.pop()
        assert popped is self._sem_poison
        self.nc.clear_and_free_semaphores(list(self.sems.allocated().values()))
        self.nc.all_engine_barrier()

    _tile.TileContext._drain_and_barrier = _patched
    _tile.TileContext._drain_patched = True


def _row_pieces(row0, nrows):
    """Split absolute rows [row0, row0+nrows) of the 672-row qkT layout into
    (tile_idx, tile_row_offset, count) pieces along 128-row tiles."""
    pieces = []
    r = row0
    while r < row0 + nrows:
        t = r // 128
        off = r % 128
        cnt = min(128 - off, row0 + nrows - r)
        pieces.append((t, off, cnt))
        r += cnt
    return pieces


def _split_dma_waits(nc):
    """This walrus build accepts only ONE sync-wait per instruction. Hoist
    excess waits onto same-engine NoOps placed just before the instruction."""
    ctr = [0]
    for fn in nc.m.functions:
        for blk in fn.blocks:
            new_list = []
            changed = False
            for inst in blk.instructions:
                si = getattr(inst, "sync_info", None)
                if si is not None and len(si.on_wait) > 1:
                    changed = True
                    waits = list(si.on_wait)
                    for w in waits[:-1]:
                        ctr[0] += 1
                        nop = mybir.InstNoOp(name=f"xw-{ctr[0]}")
                        nop.engine = inst.engine
                        nop.sync_info = mybir.SyncInfo(on_wait=[w], on_update=[])
                        new_list.append(nop)
                    si.on_wait = waits[-1:]
                    inst.sync_info = si
                new_list.append(inst)
            if changed:
                blk.instructions = new_list


def build():
    _install_tile_drain_patch()
    nc = bass.Bass()
    bf = mybir.dt.bfloat16
    f32 = mybir.dt.float32

    xt_ext = nc.declare_dram_parameter("xt", [C, T], bf, isOutput=False)
    wqk_ext = nc.declare_dram_parameter("wqk", [C, 2 * CH], bf, isOutput=False)
    wv_ext = nc.declare_dram_parameter("wv", [C, CH], bf, isOutput=False)
    wp_ext = nc.declare_dram_parameter("wp", [CH, C], bf, isOutput=False)
    # Per-key-tile causal masks replicating the reference's bf16-quantized
    # index comparison (bf16(i) >= bf16(j)) which lets queries attend up to
    # ~15 future positions near the diagonal. Mask kt covers queries
    # [128kt-128, 128kt+128) x keys [128kt, 128kt+128), laid out [128, 16*256].
    mask_ext = nc.declare_dram_parameter("qmask", [128, 16 * 256], bf, isOutput=False)
    out_ext = nc.declare_dram_parameter("out", [T, C], f32, isOutput=True)

    with tile.TileContext(nc) as tc, ExitStack() as ctx:
        pool = ctx.enter_context(tc.tile_pool(name="persist", bufs=1))
        work = ctx.enter_context(tc.tile_pool(name="work", bufs=2))
        ppool = ctx.enter_context(tc.tile_pool(name="pp", bufs=3))
        spool = ctx.enter_context(tc.tile_pool(name="spsum", bufs=2, space="PSUM"))
        opool = ctx.enter_context(tc.tile_pool(name="opsum", bufs=2, space="PSUM"))

        # ---- load inputs: interleave wqk[i] with xt[i] col-chunk 0 so the
        # first B matmul group can start after just two transfers ------------
        XT, WQK, WV = [], [], []
        for i, (k0, kn) in enumerate(KT):
            XT.append(pool.tile([kn, T], bf, tag=f"xt{i}", name=f"xt{i}"))
            WV.append(pool.tile([kn, CH], bf, tag=f"wv{i}", name=f"wv{i}"))
            WQK.append(pool.tile([kn, 2 * CH], bf, tag=f"wqk{i}", name=f"wqk{i}"))
        for i, (k0, kn) in enumerate(KT):
            nc.sync.dma_start(WQK[i][:], wqk_ext[k0 : k0 + kn, :])
            nc.sync.dma_start(XT[i][:, 0:512], xt_ext[k0 : k0 + kn, 0:512])
        for n0 in range(512, T, 512):
            for i, (k0, kn) in enumerate(KT):
                nc.sync.dma_start(
                    XT[i][:, n0 : n0 + 512], xt_ext[k0 : k0 + kn, n0 : n0 + 512]
                )
        for i, (k0, kn) in enumerate(KT):
            nc.sync.dma_start(WV[i][:], wv_ext[k0 : k0 + kn, :])
        WP = []
        for i, (k0, kn) in enumerate([(0, 128), (128, 128), (256, 80)]):
            wp_t = pool.tile([kn, C], bf, tag=f"wp{i}")
            nc.sync.dma_start(wp_t[:], wp_ext[k0 : k0 + kn, :])
            WP.append(wp_t)
        qmask = pool.tile([128, 16 * 256], bf, tag="qmask")
        nc.sync.dma_start(qmask[:], mask_ext[:])

        # ---- phase B: qkT = [wq|wk].T @ xT  -> [672, 2048] bf16 ----------
        # M-tile order puts pair 0's q (tile 0) and k (tiles 2,3) first so the
        # first attention pair can start while B finishes the rest.
        QKT = [None] * len(KT)
        for m in (0, 2, 3, 1, 4, 5):
            m0, mn = KT[m]
            qk_sb = pool.tile([mn, T], bf, tag=f"qkt{m}", name=f"qkt{m}")
            for n0 in range(0, T, 512):
                ps = spool.tile([128, 1024], f32, tag="s", name="bps")
                for ki, (k0, kn) in enumerate(KT):
                    nc.tensor.matmul(
                        ps[:mn, :512],
                        WQK[ki][:, m0 : m0 + mn],
                        XT[ki][:, n0 : n0 + 512],
                        start=(ki == 0),
                        stop=(ki == len(KT) - 1),
                    )
                nc.vector.tensor_copy(qk_sb[:, n0 : n0 + 512], ps[:mn, :512])
            QKT[m] = qk_sb

        # ---- phase C: repack per head-pair at partition bases {0, 64} ----
        # qpair[p] rows 0..48 = q of head 2p; rows 64..112 = q of head 2p+1
        QP, KP = [], []
        for p in range(4):
            qp = pool.tile([128, T], bf, tag=f"qp{p}")
            kp = pool.tile([128, T], bf, tag=f"kp{p}")
            QP.append(qp)
            KP.append(kp)
        for h in range(HPC):
            p, rb = h // 2, 64 * (h % 2)
            for dst, row0 in ((QP[p], h * D), (KP[p], CH + h * D)):
                o = 0
                for (t, off, cnt) in _row_pieces(row0, D):
                    nc.sync.dma_start(
                        dst[rb + o : rb + o + cnt, :], QKT[t][off : off + cnt, :]
                    )
                    o += cnt

        # ---- phase D: v in natural layout + ones column ------------------
        # v_aug layout: [128, HPC, 16, 49]; per (head h, key tile kt) the
        # [128, 49] slice is lhsT for AV (col 48 = ones -> denominator row).
        v_aug = pool.tile([128, HPC * 16 * 49], bf, tag="vaug")
        v4 = v_aug[:].rearrange("p (h t d) -> p h t d", h=HPC, t=16, d=49)
        nc.vector.memset(v4[:, :, :, 48:49], 1.0)
        for t in range(16):
            ps = spool.tile([128, 1024], f32, tag="s")
            for ki, (k0, kn) in enumerate(KT):
                nc.tensor.matmul(
                    ps[:, :CH],
                    XT[ki][:, t * 128 : (t + 1) * 128],
                    WV[ki][:],
                    start=(ki == 0),
                    stop=(ki == len(KT) - 1),
                )
            src3 = ps[:, :CH].rearrange("p (h d) -> p h d", h=HPC)
            nc.vector.tensor_copy(v4[:, :, t, 0:48], src3)

        # ---- phase E: attention per head-pair ----------------------------
        OPK = [pool.tile([128, T], bf, tag=f"opk{i}", name=f"opk{i}") for i in range(3)]
        Y1 = []
        for p in range(4):
            heads = [(2 * p, 0)] if p == 3 else [(2 * p, 0), (2 * p + 1, 64)]
            yh = {h: work.tile([128, T], bf, tag=f"yh{h % 2}", name=f"yh{p}")
                  for (h, _rb) in heads}
            # two query-half passes (g = 0: cols 0..1024, g = 1: cols 1024..2048)
            # so only two o banks are live at a time (allows double buffering)
            for g in range(2):
                o_ps = [
                    opool.tile([128, 512], f32, tag=f"o{half}", name=f"og{half}")
                    for half in range(2)
                ]
                kts = [kt for kt in range(16) if max(0, kt * 128 - 16) // 1024 <= g]
                for kt in kts:
                    qext = max(0, kt * 128 - 16)  # 16-col spill strip
                    ls0 = max(0, qext - g * 1024)
                    # scores for both heads issued adjacently: their lhsT live
                    # at partition bases 0/64 (row groups 0-1 vs 2-3) so the
                    # two matmuls execute concurrently in the PE array
                    p_tiles = {}
                    for (h, rb) in heads:
                        k_l = KP[p][rb : rb + D, kt * 128 : (kt + 1) * 128]
                        s_ps = spool.tile([128, 1024], f32, tag="s", name="s")
                        p_sb = ppool.tile([128, 1024], bf, tag="p", name="pp")
                        p_tiles[h] = p_sb
                        for half in range(2):
                            ls, he = max(ls0, half * 512), (half + 1) * 512
                            if ls >= he:
                                continue
                            nc.tensor.matmul(
                                s_ps[:, ls:he],
                                k_l,
                                QP[p][rb : rb + D, g * 1024 + ls : g * 1024 + he],
                                start=True,
                                stop=True,
                            )
                        nc.scalar.activation(
                            p_sb[:, ls0:1024],
                            s_ps[:, ls0:1024],
                            mybir.ActivationFunctionType.Exp,
                            scale=SCALE,
                        )
                        # masked region: global queries [128kt-128, 128kt+128)
                        mg0 = kt * 128 - 128  # global query of mask col 0
                        dls = max(ls0, mg0 - g * 1024)
                        dle = min(1024, kt * 128 + 128 - g * 1024)
                        if dls < dle:
                            mo = 256 * kt + (g * 1024 + dls - mg0)
                            nc.vector.tensor_mul(
                                p_sb[:, dls:dle],
                                p_sb[:, dls:dle],
                                qmask[:, mo : mo + (dle - dls)],
                            )
                    for (h, rb) in heads:
                        p_sb = p_tiles[h]
                        for half in range(2):
                            ls, he = max(ls0, half * 512), (half + 1) * 512
                            if ls >= he:
                                continue
                            qc = 2 * g + half
                            nc.tensor.matmul(
                                o_ps[half][rb : rb + D + 1, ls - half * 512 : 512],
                                v4[:, h, kt, :],
                                p_sb[:, ls:he],
                                start=(kt == kts[0]),
                                stop=(kt == min(4 * qc + 4, 15)),
                                tile_position=(0, rb),
                                skip_group_check=True,
                            )
                # per-pass epilogue: softmax divide for query cols of this g
                den = work.tile([128, 1024], f32, tag="den")
                for (h, rb) in heads:
                    for half in range(2):
                        # PSUM reads must start 32-partition aligned; ScalarE
                        # extracts the slab holding the denominator row (48)
                        nc.scalar.copy(
                            den[rb + 32 : rb + 64, half * 512 : (half + 1) * 512],
                            o_ps[half][rb + 32 : rb + 64, :],
                        )
                # one DVE reciprocal covers both heads' slabs (cost is
                # free-size-bound, independent of partition count; rows outside
                # the copied slabs hold garbage whose reciprocal is unused)
                nc.vector.reciprocal(den[:, :], den[:, :])
                for (h, rb) in heads:
                    # log-doubling partition broadcast of the reciprocal row
                    bc = work.tile([128, 1024], f32, tag="bc")
                    nc.sync.dma_start(
                        bc[rb : rb + 1, :], den[rb + D : rb + D + 1, :]
                    )
                    filled = 1
                    while filled < D:
                        n = min(filled, D - filled)
                        nc.sync.dma_start(
                            bc[rb + filled : rb + filled + n, :], bc[rb : rb + n, :]
                        )
                        filled += n
                    for half in range(2):
                        nc.vector.tensor_mul(
                            yh[h][rb : rb + D, (2 * g + half) * 512 : (2 * g + half + 1) * 512],
                            o_ps[half][rb : rb + D, :],
                            bc[rb : rb + D, half * 512 : (half + 1) * 512],
                        )
            for (h, rb) in heads:
                o = 0
                for (t, off, cnt) in _row_pieces(h * D, D):
                    nc.sync.dma_start(
                        OPK[t][off : off + cnt, :], yh[h][rb + o : rb + o + cnt, :]
                    )
                    o += cnt

        # ---- phase F: y = opk.T @ wp -> out [2048, 672] ------------------
        PKT = [(0, 128), (128, 128), (256, 80)]
        for t in range(16):
            ysb = work.tile([128, C], f32, tag="ysb")
            for n0, nn in ((0, 512), (512, 160)):
                ps = spool.tile([128, 1024], f32, tag="s", name="fps")
                for ki, (k0, kn) in enumerate(PKT):
                    nc.tensor.matmul(
                        ps[:, :nn],
                        OPK[ki][:kn, t * 128 : (t + 1) * 128],
                        WP[ki][:, n0 : n0 + nn],
                        start=(ki == 0),
                        stop=(ki == 2),
                    )
                nc.vector.tensor_copy(ysb[:, n0 : n0 + nn], ps[:, :nn])
            nc.sync.dma_start(out_ext[t * 128 : (t + 1) * 128, :], ysb[:])

    _split_dma_waits(nc)
    return nc


_NC_CACHE = None


def _get_nc():
    global _NC_CACHE
    if _NC_CACHE is None:
        _NC_CACHE = build()
    return _NC_CACHE


def make_in_maps(x, w_attn, w_proj):
    # bf16-quantized causal masks, one [128, 256] block per key tile kt:
    # mask[j - 128kt, i - (128kt - 128)] = bf16(i) >= bf16(j)
    idx = np.arange(T, dtype=np.float32).astype(BF16).astype(np.float32)
    qm = np.zeros((128, 16 * 256), dtype=np.float32)
    for kt in range(16):
        jg = idx[kt * 128 : (kt + 1) * 128]
        i0 = kt * 128 - 128
        ig = np.where(
            (np.arange(i0, i0 + 256) >= 0) & (np.arange(i0, i0 + 256) < T),
            idx[np.clip(np.arange(i0, i0 + 256), 0, T - 1)],
            -1.0,
        )
        qm[:, kt * 256 : (kt + 1) * 256] = (ig[None, :] >= jg[:, None]).astype(
            np.float32
        )
    qmask = qm.astype(BF16)
    in_maps = []
    for c in range(8):
        b, s = c // 2, c % 2
        xt = np.ascontiguousarray(x[b].T).astype(BF16)
        wq = w_attn[:, s * CH : (s + 1) * CH]
        wk = w_attn[:, C + s * CH : C + (s + 1) * CH]
        wv = w_attn[:, 2 * C + s * CH : 2 * C + (s + 1) * CH]
        wqk = np.concatenate([wq, wk], axis=1).astype(BF16)
        wp = w_proj[s * CH : (s + 1) * CH, :].astype(BF16)
        in_maps.append(
            {
                "xt": xt,
                "wqk": np.ascontiguousarray(wqk),
                "wv": np.ascontiguousarray(wv.astype(BF16)),
                "wp": np.ascontiguousarray(wp),
                "qmask": qmask,
            }
        )
    return in_maps


def run(x, w_attn, w_proj, trace=False):
    nc = _get_nc()
    in_maps = make_in_maps(x, w_attn, w_proj)
    res = run_bass_kernel_spmd(nc, in_maps, core_ids=list(range(8)), trace=trace)
    parts = [res.results[c]["out"] for c in range(8)]
    y = np.stack([parts[2 * b] + parts[2 * b + 1] for b in range(B)], axis=0)
    return y.astype(BF16), res


def kernel(x, w_attn, w_proj):
    y, _ = run(np.asarray(x, dtype=np.float32),
               np.asarray(w_attn, dtype=np.float32),
               np.asarray(w_proj, dtype=np.float32))
    return y

